# revision 9
# baseline (speedup 1.0000x reference)
"""Multi-head attention (ReLU-gated projections) on 8 Trainium2 NeuronCores.

Problem (hardcoded): B=4, S=1024, H=1024, NH=16, DH=64.
  qp = relu(q @ Wq.T + bq); kp, vp likewise
  alpha = softmax(qh @ kh.T / sqrt(DH)) * mask[q]
  out = (alpha @ vh).reshape(B,S,H) + query

Sharding: 8 cores = 4 batches x 2 head-groups (8 heads / 512 hidden cols each).

fp8 design: all matmuls in fp8 e4m3 (TRN2 flavor: with-inf, max finite 240).
Weights pre-scaled by 32 on the host so their N(0, 1/32) values use e4m3's
normal range; the 32x factors ride through the linear pipeline (qp,kp,vp
all carry 32x) and are compensated in the exp scale (1/(8*32^2)) and a
final /32 on the host. exp also subtracts 3.0 (cancels in softmax) to
keep pt under the 240 cap. Projections and AV use MatmulPerfMode.DoubleRow
(K=256 per instruction, 2x PE throughput); alpha matmuls are
output-rate-bound so they stay plain fp8 with the kz zero-padded-K trick.
The AV stationary keeps a ones column (M=65) so row 64 accumulates sumexp
for free; the per-head V slot is padded to 68 bytes so DoubleRow weight
APs stay 4-byte aligned (ISA restriction s3_lw_dual_fp8).

Host pre-arranges x/w into the exact SBUF layouts so every input DMA is
128 partitions x 4KB contiguous.

Schedule: the ACT exp stream is the metronome (64 exp tiles ~1.1us each >
PE's ~63us of matmul work), so the kernel is organized to start that
stream as early as possible and never stall it:
  - DMA rings are priority-ordered (wq+xq0 first, then wk/xk0/xq1) so the
    first q/k projections start as soon as ~1.5MB have landed.
  - Head 0's first alpha tiles are emitted as qc-split HALF tiles
    [128,512] so the first exp only needs q-proj(sc0)+k-proj(sc0) --
    about 6us earlier than waiting for the full-S qp tile.
  - The critical first kz evacuation is split across ACT (h0, Relu
    activation) and DVE (h1) so the two halves land in parallel.
  - All other PE work (projections, AV chunks) is queued as fill units
    popped between alpha tiles, ordered so no fill is emitted on PE
    before its producers (q before k before alpha; all v before any AV).
  - The final hid DMA is split into 256-col chunks across both DMA rings
    so the run doesn't tail-wait on one 130KB transfer.
"""
import sys

sys.path.insert(0, "/opt/trn_rl_repo")

import os
import numpy as np
import ml_dtypes

import concourse.bass as bass
import concourse.tile as tile
from concourse import bacc, mybir
from concourse import bass_utils

B, S, H = 4, 1024, 1024
NH, DH = 16, 64
NCORES = 8
GROUPS = 2          # head-groups (tensor-parallel dim)
HL = NH // GROUPS   # heads per core = 8
GH = H // GROUPS    # hidden cols per core = 512
KT = H // 128       # contraction k-tiles = 8
OT = GH // 128      # output o-tiles per core = 4
SCALE = 1.0 / float(np.sqrt(DH))
ESC = 32.0          # fp8 weight pre-scale (TRN2 fp8e4 = e4m3-with-inf,
                    # max finite 240: keep relu'd projections under ~170)
VW8 = HL * 68       # padded per-head v slot (64 v + 1 ones + 3 pad) = 544

MODE = os.environ.get("BASS_MM_DT", "fp8")

F32 = mybir.dt.float32
BF16 = mybir.dt.bfloat16
FP8 = mybir.dt.float8e4
DR = mybir.MatmulPerfMode.DoubleRow
E4 = ml_dtypes.float8_e4m3   # e4m3 WITH inf (max 240) — matches TRN2 hw


def build_fp8():
    nc = bacc.Bacc("TRN2", target_bir_lowering=False, debug=False,
                   num_devices=NCORES)

    # x/w arrive pre-arranged in SBUF layout: [128, KT*512] per s-chunk
    x_d = {(w, sc): nc.dram_tensor(f"x{w}{sc}", [128, KT * 512], FP8,
                                   kind="ExternalInput").ap()
           for w in "qkv" for sc in range(2)}
    w_d = {w: nc.dram_tensor(f"w{w}", [128, KT * GH], FP8,
                             kind="ExternalInput").ap()
           for w in "qkv"}
    bqk_d = nc.dram_tensor("bqk", [128, 2 * OT], F32, kind="ExternalInput").ap()
    bv_d = nc.dram_tensor("bv", [1, GH], FP8, kind="ExternalInput").ap()
    ones_d = nc.dram_tensor("onesd", [128, 128], FP8,
                            kind="ExternalInput").ap()
    hid_d = nc.dram_tensor("hid", [HL * (DH + 1), S], F32,
                           kind="ExternalOutput").ap()

    EXP_SCALE = SCALE / (ESC * ESC)
    EXP_BIAS = -3.0   # pt = exp(alpha/8 - 3): keeps exp under e4m3 max 240;
                      # cancels in hid/sumexp

    with tile.TileContext(nc) as tc:
        with tc.tile_pool(name="sb", bufs=1) as sb, \
             tc.tile_pool(name="ps", bufs=1, space="PSUM") as ps:

            # ---- persistent tiles ----
            wq_t = sb.tile([128, KT * GH], FP8, tag="wq", name="wq")
            wk_t = sb.tile([128, KT * GH], FP8, tag="wk", name="wk")
            wv_t = sb.tile([128, KT * GH], FP8, tag="wv", name="wv")
            qp_t = [sb.tile([128, S], FP8, tag=f"qp{t}", name=f"qp{t}")
                    for t in range(OT)]
            kz_t = [[sb.tile([128, S], FP8, tag=f"kz{t}{h}",
                             name=f"kz{t}{h}") for h in range(2)]
                    for t in range(OT)]
            kz_zeroed = set()
            vp_t = sb.tile([128, KT * VW8], FP8, tag="vp", name="vp")
            bqk_t = sb.tile([128, 2 * OT], F32, tag="bqk", name="bqk")
            bv_t = sb.tile([1, GH], FP8, tag="bv", name="bv")
            ones_t = sb.tile([1, 128], FP8, tag="ones", name="ones")
            ones64_t = sb.tile([128, KT * HL], FP8, tag="ones64",
                               name="ones64")
            expb_t = sb.tile([128, 1], F32, tag="expb", name="expb")
            nc.vector.memset(expb_t[:], EXP_BIAS)

            # ---- warmup from memset tiles: no DMA dependency, so the PE
            #      clock ramp and the ACT exp-table preload start at ~1us ----
            wstat = sb.tile([1, 128], FP8, tag="wstat", name="wstat")
            wmov = sb.tile([1, 512], FP8, tag="wmov", name="wmov")
            nc.vector.memset(wstat[:], 1.0)
            nc.vector.memset(wmov[:], 1.0)
            warm = ps.tile([65, 512], F32, tag="av", bufs=2, name="warm")
            for i in range(7):
                nc.tensor.matmul(warm[:], wstat[:, 0:65], wmov[:],
                                 start=True, stop=True)
            dummy_exp = sb.tile([1, 8], F32, tag="dummy_exp", name="dummy_exp")
            nc.scalar.activation(dummy_exp[:], wmov[0:1, 0:8],
                                 mybir.ActivationFunctionType.Exp, scale=1.0)

            # ---- loads: whole tiles (4KB contiguous runs), three rings in
            #      parallel, priority-ordered: the first alpha needs
            #      wq+xq0 (q proj sc0) then wk+xk0 (k proj sc0); xq1 next
            #      (qc1 half-alphas), then xk1, then everything v ----
            x_t = {}
            ring_i = [0]

            def x_ld(which, sc, eng):
                t = sb.tile([128, KT * 512], FP8, tag=f"x{which}{sc}",
                            name=f"x{which}_{sc}")
                x_t[(which, sc)] = t
                eng.dma_start(t[:], x_d[(which, sc)])

            def x3(which, sc):
                return x_t[(which, sc)][:].rearrange("p (k s) -> p k s", s=512)

            # Each DMA ring is a SERIAL queue at ~110GB/s; the three rings
            # run in parallel.  Critical tensors are split into k-halves
            # spread across rings, ordered by first consumption: the DR
            # proj chains read k-slices in order, so part0 of the first
            # q-projection starts once wq.h0+xq0.h0 (512KB) have landed.
            HC = KT * 512 // 2   # 2048 cols = k-slices 0-3

            def x_ld_half(which, sc, eng, h):
                if (which, sc) not in x_t:
                    x_t[(which, sc)] = sb.tile(
                        [128, KT * 512], FP8, tag=f"x{which}{sc}",
                        name=f"x{which}_{sc}")
                t = x_t[(which, sc)]
                eng.dma_start(t[:, h * HC:(h + 1) * HC],
                              x_d[(which, sc)][:, h * HC:(h + 1) * HC])

            # sync ring
            x_ld_half("q", 0, nc.sync, 0)
            x_ld_half("q", 0, nc.sync, 1)
            nc.sync.dma_start(wk_t[:, 0:HC], w_d["k"][:, 0:HC])
            x_ld_half("q", 1, nc.sync, 0)
            x_ld("v", 0, nc.sync)
            nc.sync.dma_start(ones64_t[:], ones_d[:, 0:KT * HL])
            # gpsimd ring
            nc.gpsimd.dma_start(wq_t[:, 0:HC], w_d["q"][:, 0:HC])
            nc.gpsimd.dma_start(wq_t[:, HC:2 * HC], w_d["q"][:, HC:2 * HC])
            nc.gpsimd.dma_start(wk_t[:, HC:2 * HC], w_d["k"][:, HC:2 * HC])
            x_ld_half("q", 1, nc.gpsimd, 1)
            # zero the kz pads for o-tile 0: needed before the first alpha,
            # the gpsimd engine is free once its descriptors are out
            nc.gpsimd.memset(kz_t[0][0][64:128, :], 0.0)
            nc.gpsimd.memset(kz_t[0][1][0:64, :], 0.0)
            kz_zeroed.add(0)
            nc.gpsimd.dma_start(bv_t[:], bv_d)
            nc.gpsimd.dma_start(ones_t[:], ones_d[0:1, :])
            nc.gpsimd.dma_start(wv_t[:], w_d["v"])
            # scalar ring (descriptors only; they issue before the ACT
            # stream starts and the transfers queue serially)
            nc.scalar.dma_start(bqk_t[:], bqk_d)
            x_ld_half("k", 0, nc.scalar, 0)
            x_ld_half("k", 0, nc.scalar, 1)
            x_ld_half("k", 1, nc.scalar, 0)
            x_ld_half("k", 1, nc.scalar, 1)
            x_ld("v", 1, nc.scalar)

            v4 = vp_t[:].rearrange("p (k n c) -> p k n c", n=HL, c=68)

            pp_live = {}

            def proj_qk(sc, ot, which, part=None, act_evac=False):
                """one o-tile, one s-chunk of the transposed q/k projection;
                part 0/1 emit half the DR chain each (fill-unit sizing),
                part None emits the whole group.  act_evac: for k, evacuate
                the h0 half on ACT (idle pre-stream) in parallel with DVE."""
                w_t = wq_t if which == "q" else wk_t
                w3 = w_t[:].rearrange("p (k o) -> p k o", o=GH)
                xv_ = x3(which, sc)
                if part != 1:
                    pp_live[(sc, ot, which)] = ps.tile(
                        [128, 1024], F32, tag="alpha", bufs=3,
                        name=f"pp{which}_{sc}_{ot}")
                pp = pp_live[(sc, ot, which)]
                kps = range(KT // 2) if part is None else (
                    range(2) if part == 0 else range(2, 4))
                for kp in kps:
                    nc.tensor.matmul(
                        pp[:, 0:512],
                        w3[:, 2 * kp:2 * kp + 2, ot * 128:(ot + 1) * 128],
                        xv_[:, 2 * kp:2 * kp + 2, :],
                        start=(kp == 0), stop=(kp == KT // 2 - 1),
                        perf_mode=DR)
                if part == 0:
                    return
                wi = 0 if which == "q" else 1
                bias = bqk_t[:, wi * OT + ot:wi * OT + ot + 1]
                ssl = slice(sc * 512, (sc + 1) * 512)
                if which == "q":
                    nc.vector.tensor_scalar(
                        qp_t[ot][:, ssl], pp[:, 0:512], bias, 0.0,
                        mybir.AluOpType.add, mybir.AluOpType.max)
                else:
                    for h in range(2):
                        pr = slice(h * 64, h * 64 + 64)
                        if h == 0 and act_evac:
                            # critical path: ACT is idle before the exp
                            # stream; do h0 there while DVE does h1
                            nc.scalar.activation(
                                kz_t[ot][h][pr, ssl], pp[pr, 0:512],
                                mybir.ActivationFunctionType.Relu,
                                bias=bias[pr, :], scale=1.0)
                        else:
                            nc.vector.tensor_scalar(
                                kz_t[ot][h][pr, ssl], pp[pr, 0:512],
                                bias[pr, :], 0.0,
                                mybir.AluOpType.add, mybir.AluOpType.max)
                pp_live.pop((sc, ot, which))

            def proj_v(sc, j, part=None):
                """one s-tile (128 rows of vp) within chunk sc"""
                st = sc * 4 + j
                wv3 = wv_t[:].rearrange("p (k o) -> p k o", o=GH)
                xv_ = x3("v", sc)
                if part != 1:
                    pp_live[("v", st)] = ps.tile([128, 1024], F32,
                                                 tag="alpha", bufs=3,
                                                 name=f"ppv_{st}")
                    nc.tensor.matmul(pp_live[("v", st)][:, 0:512],
                                     ones_t[:], bv_t[:],
                                     start=True, stop=False)
                pp = pp_live[("v", st)]
                kps = range(KT // 2) if part is None else (
                    range(2) if part == 0 else range(2, 4))
                for kp in kps:
                    nc.tensor.matmul(
                        pp[:, 0:512],
                        xv_[:, 2 * kp:2 * kp + 2, j * 128:(j + 1) * 128],
                        wv3[:, 2 * kp:2 * kp + 2, :],
                        start=False, stop=(kp == KT // 2 - 1),
                        perf_mode=DR)
                if part == 0:
                    return
                v3 = vp_t[:, st * VW8:(st + 1) * VW8].rearrange(
                    "p (n c) -> p n c", c=68)
                p3 = pp[:, 0:512].rearrange("p (n c) -> p n c", c=DH)
                nc.vector.tensor_scalar(
                    v3[:, :, 0:DH], p3, 0.0, None, mybir.AluOpType.max)
                pp_live.pop(("v", st))

            pt_all = {}
            fill_q = []

            def a_tile(n, k, qc=None):
                """alpha matmul(s) + exp for head n, k-tile k.  qc None:
                full [128,1024] tile (one exp).  qc 0/1: a HALF tile --
                only needs qp s-chunk qc, so the first exps can fire
                before xq1/q-proj(sc1) are done."""
                t, h = n // 2, n % 2
                pair, half = k // 2, k % 2
                pts = pt_all.setdefault(n, [None] * (KT // 2))
                if pts[pair] is None:
                    pts[pair] = sb.tile([128, 2048], FP8, tag="pt",
                                        bufs=32, name=f"pt_{n}_{pair}")
                cur = pts[pair]
                apt = ps.tile([128, 1024], F32, tag="alpha", bufs=3,
                              name=f"alp_{n}_{k}_{qc}")
                qcs = (0, 1) if qc is None else (qc,)
                for q_ in qcs:
                    nc.tensor.matmul(
                        apt[:, q_ * 512:(q_ + 1) * 512],
                        kz_t[t][h][:, k * 128:(k + 1) * 128],
                        qp_t[t][:, q_ * 512:(q_ + 1) * 512],
                        start=True, stop=True)
                if qc is None:
                    nc.scalar.activation(
                        cur[:, half * 1024:(half + 1) * 1024], apt[:],
                        mybir.ActivationFunctionType.Exp, scale=EXP_SCALE,
                        bias=expb_t[:])
                else:
                    off = half * 1024 + qc * 512
                    nc.scalar.activation(
                        cur[:, off:off + 512], apt[:, qc * 512:qc * 512 + 512],
                        mybir.ActivationFunctionType.Exp, scale=EXP_SCALE,
                        bias=expb_t[:])

            def alphas_h(n, pops=()):
                """alpha + exp for one head, full tiles, popping fill units
                from fill_q between tiles so the PE's ACT-limited stall
                time does the projections and AV chunks."""
                t = n // 2
                if t not in kz_zeroed:
                    kz_zeroed.add(t)
                    nc.gpsimd.memset(kz_t[t][0][64:128, :], 0.0)
                    nc.gpsimd.memset(kz_t[t][1][0:64, :], 0.0)
                for k in range(KT):
                    a_tile(n, k)
                    if k in pops and fill_q:
                        fill_q.pop(0)()

            hid_tiles = {}
            av_live = {}

            def avs_qc(n, qc, last=False, part=None):
                pts = pt_all[n]
                if qc == 0 and part != 1:
                    hid_tiles[n] = sb.tile([DH + 1, S], F32, tag="hid",
                                           bufs=3, name=f"hid_{n}")
                hid_t = hid_tiles[n]
                if part != 1:
                    av_live[(n, qc)] = ps.tile([DH + 1, 512], F32, tag="av",
                                               bufs=2, name=f"av_{n}_{qc}")
                av = av_live[(n, qc)]
                kps = range(KT // 2) if part is None else (
                    range(2) if part == 0 else range(2, 4))
                for kp in kps:
                    nc.tensor.matmul(
                        av[:],
                        v4[:, 2 * kp:2 * kp + 2, n, 0:DH + 1],
                        pts[kp][:].rearrange(
                            "p (k s) -> p k s",
                            s=1024)[:, :, qc * 512:(qc + 1) * 512],
                        start=(kp == 0), stop=(kp == KT // 2 - 1),
                        perf_mode=DR)
                if part == 0:
                    return
                av_live.pop((n, qc))
                if last:
                    # ACT is idle after its final exp — use it so the two
                    # tail evacuations run on different engines
                    nc.scalar.copy(
                        hid_t[:, qc * 512:(qc + 1) * 512], av[:])
                else:
                    nc.vector.tensor_copy(
                        hid_t[:, qc * 512:(qc + 1) * 512], av[:])
                # never the scalar ring: a DMA descriptor op there would
                # steal ~0.8us from the ACT exp stream
                if last:
                    # split the tail transfer across both rings so the run
                    # doesn't end waiting on one 130KB DMA
                    for ci, eng in ((0, nc.sync), (1, nc.gpsimd)):
                        cs = slice(qc * 512 + ci * 256, qc * 512 + ci * 256 + 256)
                        eng.dma_start(
                            hid_d[n * (DH + 1):(n + 1) * (DH + 1), cs],
                            hid_t[:, cs])
                else:
                    eng = nc.sync if ring_i[0] % 2 == 0 else nc.gpsimd
                    ring_i[0] += 1
                    eng.dma_start(
                        hid_d[n * (DH + 1):(n + 1) * (DH + 1),
                              qc * 512:(qc + 1) * 512],
                        hid_t[:, qc * 512:(qc + 1) * 512])
                if qc == 1:
                    pt_all.pop(n)
                    hid_tiles.pop(n)

            def u2(f, *a):
                fill_q.append(lambda: f(*a, part=0))
                fill_q.append(lambda: f(*a, part=1))

            # ---- early phase: minimize time to the first exp.  q(0,0)
            #      needs only wq+xq0 (first DMAs); k(0,0) needs wk+xk0
            #      (second wave).  The first four alpha tiles are qc0-only
            #      halves so they don't wait for xq1/q-proj(sc1). ----
            proj_qk(0, 0, "q")
            proj_qk(0, 0, "k", act_evac=True)
            a_tile(0, 0, qc=0)
            a_tile(0, 1, qc=0)
            a_tile(0, 2, qc=0)
            a_tile(0, 3, qc=0)
            proj_qk(1, 0, "q")
            proj_qk(1, 0, "k", act_evac=True)
            a_tile(0, 0, qc=1)
            a_tile(0, 1, qc=1)
            a_tile(0, 2, qc=1)
            a_tile(0, 3, qc=1)
            a_tile(0, 4)
            a_tile(0, 5)
            # ones column of the AV stationary (needs ones64; first AV
            # unit pops much later, and the early DVE evacuations are done)
            nc.vector.tensor_copy(
                v4[:, :, :, DH:DH + 1],
                ones64_t[:].rearrange("p (k n one) -> p k n one", n=HL, one=1))
            proj_qk(0, 1, "q")
            a_tile(0, 6)
            a_tile(0, 7)

            # ---- steady state: head-at-a-time alpha/exp stream with fill
            #      units.  Each o-tile's four remaining projections drain
            #      two heads ahead of the alphas that read them; all eight
            #      v s-tiles drain before the first AV unit. ----
            u2(proj_qk, 0, 1, "k")
            u2(proj_qk, 1, 1, "q")
            u2(proj_qk, 1, 1, "k")
            alphas_h(1, pops=(1, 2, 3, 4, 5, 6))
            u2(proj_qk, 0, 2, "q")
            u2(proj_qk, 0, 2, "k")
            u2(proj_qk, 1, 2, "q")
            u2(proj_qk, 1, 2, "k")
            alphas_h(2, pops=(0, 1, 2, 3, 4, 5, 6, 7))
            u2(proj_v, 0, 0)
            u2(proj_v, 0, 1)
            u2(proj_v, 0, 2)
            u2(proj_v, 0, 3)
            alphas_h(3, pops=(0, 1, 2, 3, 4, 5, 6, 7))
            u2(proj_qk, 0, 3, "q")
            u2(proj_qk, 0, 3, "k")
            u2(proj_qk, 1, 3, "q")
            u2(proj_qk, 1, 3, "k")
            alphas_h(4, pops=(0, 1, 2, 3, 4, 5, 6, 7))
            u2(proj_v, 1, 0)
            u2(proj_v, 1, 1)
            u2(proj_v, 1, 2)
            u2(proj_v, 1, 3)
            alphas_h(5, pops=(0, 1, 2, 3, 4, 5, 6, 7))
            u2(avs_qc, 0, 0)
            u2(avs_qc, 0, 1)
            u2(avs_qc, 1, 0)
            u2(avs_qc, 1, 1)
            alphas_h(6, pops=(0, 1, 2, 3, 4, 5, 6, 7))
            u2(avs_qc, 2, 0)
            u2(avs_qc, 2, 1)
            u2(avs_qc, 3, 0)
            u2(avs_qc, 3, 1)
            u2(avs_qc, 4, 0)
            alphas_h(7, pops=(1, 2, 3, 4, 5, 6, 7))
            while fill_q:
                fill_q.pop(0)()
            avs_qc(4, 1)
            avs_qc(5, 0)
            avs_qc(5, 1)
            avs_qc(6, 0)
            avs_qc(6, 1)
            # head 7's first AV halves use pt pairs 0-1 (ready ~4 exps early);
            # only the two closing 2-DR chains depend on the final exp
            avs_qc(7, 0, part=0)
            avs_qc(7, 1, part=0)
            avs_qc(7, 0, part=1)
            avs_qc(7, 1, part=1, last=True)

    nc.compile()
    return nc


_NC_CACHE = {}


def _get_nc(mode):
    if mode not in _NC_CACHE:
        if mode != "fp8":
            raise ValueError(f"unsupported mode {mode}")
        _NC_CACHE[mode] = build_fp8()
    return _NC_CACHE[mode]


def _sbuf_layout_x(xT):
    """[H, S] transposed input -> per-chunk [128, KT*512] SBUF image"""
    x4 = xT.reshape(KT, 128, 2, 512)          # [k, p, sc, s]
    return [np.ascontiguousarray(
        x4[:, :, sc, :].transpose(1, 0, 2).reshape(128, KT * 512)).astype(E4)
        for sc in range(2)]


def _sbuf_layout_w(wT):
    """[H, GH] transposed weight -> [128, KT*GH] SBUF image"""
    w3 = wT.reshape(KT, 128, GH)
    return np.ascontiguousarray(
        w3.transpose(1, 0, 2).reshape(128, KT * GH)).astype(E4)


def _prep_inputs(inputs):
    q = np.asarray(inputs["query"], np.float32)
    k = np.asarray(inputs["key"], np.float32)
    v = np.asarray(inputs["value"], np.float32)
    Wq = np.asarray(inputs["Wq"], np.float32)
    Wk = np.asarray(inputs["Wk"], np.float32)
    Wv = np.asarray(inputs["Wv"], np.float32)
    bq = np.asarray(inputs["bq"], np.float32)
    bk = np.asarray(inputs["bk"], np.float32)
    bv = np.asarray(inputs["bv"], np.float32)

    xq = [_sbuf_layout_x(q[b].T) for b in range(B)]
    xk = [_sbuf_layout_x(k[b].T) for b in range(B)]
    xv = [_sbuf_layout_x(v[b].T) for b in range(B)]
    in_maps = []
    for c in range(NCORES):
        b, g = c // GROUPS, c % GROUPS
        sl = slice(g * GH, (g + 1) * GH)
        bqk = np.stack([(ESC * bq[sl]).reshape(OT, 128).T,
                        (ESC * bk[sl]).reshape(OT, 128).T],
                       1).reshape(128, 2 * OT)
        in_maps.append({
            "xq0": xq[b][0], "xq1": xq[b][1],
            "xk0": xk[b][0], "xk1": xk[b][1],
            "xv0": xv[b][0], "xv1": xv[b][1],
            "wq": _sbuf_layout_w(ESC * Wq[sl, :].T),
            "wk": _sbuf_layout_w(ESC * Wk[sl, :].T),
            "wv": _sbuf_layout_w(ESC * Wv[sl, :].T),
            "bqk": np.ascontiguousarray(bqk, dtype=np.float32),
            "bv": np.ascontiguousarray(ESC * bv[None, sl]).astype(E4),
            "onesd": np.ones((128, 128), E4),
        })
    return in_maps


def run(inputs, mode=MODE, trace=False):
    nc = _get_nc(mode)
    in_maps = _prep_inputs(inputs)
    res = bass_utils.run_bass_kernel_spmd(
        nc, in_maps, core_ids=list(range(NCORES)), trace=trace)

    masks = np.asarray(inputs["masks"], np.float32)
    query = np.asarray(inputs["query"], np.float32)
    out = np.empty((B, S, H), np.float32)
    for c in range(NCORES):
        b, g = c // GROUPS, c % GROUPS
        hid = res.results[c]["hid"].reshape(HL, DH + 1, S)
        hT = hid[:, :DH, :]                      # (HL, DH, S)  (32x scaled)
        se = hid[:, DH, :]                       # (HL, S)
        blk = (hT / (ESC * se[:, None, :])).transpose(2, 0, 1).reshape(S, GH)
        out[b, :, g * GH:(g + 1) * GH] = blk
    out = out * masks[:, :, None] + query
    return out, res


def kernel(**inputs) -> np.ndarray:
    out, _ = run(inputs)
    return out


# revision 12
# speedup vs baseline: 1.0468x; 1.0468x over previous
"""Multi-head attention (ReLU-gated projections) on 8 Trainium2 NeuronCores.

Problem (hardcoded): B=4, S=1024, H=1024, NH=16, DH=64.
  qp = relu(q @ Wq.T + bq); kp, vp likewise
  alpha = softmax(qh @ kh.T / sqrt(DH)) * mask[q]
  out = (alpha @ vh).reshape(B,S,H) + query

Sharding: 8 cores = 4 batches x 2 head-groups (8 heads / 512 hidden cols each).

fp8 design: all matmuls in fp8 e4m3 (TRN2 flavor: with-inf, max finite 240).
Weights pre-scaled by 32 on the host so their N(0, 1/32) values use e4m3's
normal range; the 32x factors ride through the linear pipeline (qp,kp,vp
all carry 32x) and are compensated in the exp scale (1/(8*32^2)) and a
final /32 on the host. exp also subtracts 3.0 (cancels in softmax) to
keep pt under the 240 cap. Projections and AV use MatmulPerfMode.DoubleRow
(K=256 per instruction, 2x PE throughput); alpha matmuls are
output-rate-bound so they stay plain fp8 with the kz zero-padded-K trick.
The AV stationary keeps a ones column (M=65) so row 64 accumulates sumexp
for free; the per-head V slot is padded to 68 bytes so DoubleRow weight
APs stay 4-byte aligned (ISA restriction s3_lw_dual_fp8).

Host pre-arranges x/w into the exact SBUF layouts so every input DMA is
128 partitions x 4KB contiguous.

Schedule: the ACT exp stream is the metronome (64 exp tiles ~1.1us each >
PE's ~63us of matmul work), so the kernel is organized to start that
stream as early as possible and never stall it:
  - DMA rings are priority-ordered (wq+xq0 first, then wk/xk0/xq1) so the
    first q/k projections start as soon as ~1.5MB have landed.
  - Head 0's first alpha tiles are emitted as qc-split HALF tiles
    [128,512] so the first exp only needs q-proj(sc0)+k-proj(sc0) --
    about 6us earlier than waiting for the full-S qp tile.
  - The critical first kz evacuation is split across ACT (h0, Relu
    activation) and DVE (h1) so the two halves land in parallel.
  - All other PE work (projections, AV chunks) is queued as fill units
    popped between alpha tiles, ordered so no fill is emitted on PE
    before its producers (q before k before alpha; all v before any AV).
  - The final hid DMA is split into 256-col chunks across both DMA rings
    so the run doesn't tail-wait on one 130KB transfer.
"""
import sys

sys.path.insert(0, "/opt/trn_rl_repo")

import os
import numpy as np
import ml_dtypes

import concourse.bass as bass
import concourse.tile as tile
from concourse import bacc, mybir
from concourse import bass_utils

B, S, H = 4, 1024, 1024
NH, DH = 16, 64
NCORES = 8
GROUPS = 2          # head-groups (tensor-parallel dim)
HL = NH // GROUPS   # heads per core = 8
GH = H // GROUPS    # hidden cols per core = 512
KT = H // 128       # contraction k-tiles = 8
OT = GH // 128      # output o-tiles per core = 4
SCALE = 1.0 / float(np.sqrt(DH))
ESC = 32.0          # fp8 weight pre-scale (TRN2 fp8e4 = e4m3-with-inf,
                    # max finite 240: keep relu'd projections under ~170)
VW8 = HL * 68       # padded per-head v slot (64 v + 1 ones + 3 pad) = 544

MODE = os.environ.get("BASS_MM_DT", "fp8")

F32 = mybir.dt.float32
BF16 = mybir.dt.bfloat16
FP8 = mybir.dt.float8e4
DR = mybir.MatmulPerfMode.DoubleRow
E4 = ml_dtypes.float8_e4m3   # e4m3 WITH inf (max 240) — matches TRN2 hw


def build_fp8():
    nc = bacc.Bacc("TRN2", target_bir_lowering=False, debug=False,
                   num_devices=NCORES)

    # x/w arrive pre-arranged in SBUF layout: [128, KT*512] per s-chunk
    x_d = {(w, sc): nc.dram_tensor(f"x{w}{sc}", [128, KT * 512], FP8,
                                   kind="ExternalInput").ap()
           for w in "qkv" for sc in range(2)}
    w_d = {w: nc.dram_tensor(f"w{w}", [128, KT * GH], FP8,
                             kind="ExternalInput").ap()
           for w in "qkv"}
    bqk_d = nc.dram_tensor("bqk", [128, 2 * OT], F32, kind="ExternalInput").ap()
    bv_d = nc.dram_tensor("bv", [1, GH], FP8, kind="ExternalInput").ap()
    ones_d = nc.dram_tensor("onesd", [128, 128], FP8,
                            kind="ExternalInput").ap()
    hid_d = nc.dram_tensor("hid", [HL * (DH + 1), S], F32,
                           kind="ExternalOutput").ap()

    EXP_SCALE = SCALE / (ESC * ESC)
    EXP_BIAS = -3.0   # pt = exp(alpha/8 - 3): keeps exp under e4m3 max 240;
                      # cancels in hid/sumexp

    with tile.TileContext(nc) as tc:
        with tc.tile_pool(name="sb", bufs=1) as sb, \
             tc.tile_pool(name="ps", bufs=1, space="PSUM") as ps:

            # ---- persistent tiles ----
            wq_t = sb.tile([128, KT * GH], FP8, tag="wq", name="wq")
            wk_t = sb.tile([128, KT * GH], FP8, tag="wk", name="wk")
            wv_t = sb.tile([128, KT * GH], FP8, tag="wv", name="wv")
            qp_t = [sb.tile([128, S], FP8, tag=f"qp{t}", name=f"qp{t}")
                    for t in range(OT)]
            kz_t = [[sb.tile([128, S], FP8, tag=f"kz{t}{h}",
                             name=f"kz{t}{h}") for h in range(2)]
                    for t in range(OT)]
            kz_zeroed = set()
            vp_t = sb.tile([128, KT * VW8], FP8, tag="vp", name="vp")
            bqk_t = sb.tile([128, 2 * OT], F32, tag="bqk", name="bqk")
            bv_t = sb.tile([1, GH], FP8, tag="bv", name="bv")
            ones_t = sb.tile([1, 128], FP8, tag="ones", name="ones")
            ones64_t = sb.tile([128, KT * HL], FP8, tag="ones64",
                               name="ones64")
            expb_t = sb.tile([128, 1], F32, tag="expb", name="expb")
            nc.vector.memset(expb_t[:], EXP_BIAS)

            # ---- warmup from memset tiles: no DMA dependency, so the PE
            #      clock ramp and the ACT exp-table preload start at ~1us ----
            wstat = sb.tile([1, 128], FP8, tag="wstat", name="wstat")
            wmov = sb.tile([1, 512], FP8, tag="wmov", name="wmov")
            nc.vector.memset(wstat[:], 1.0)
            nc.vector.memset(wmov[:], 1.0)
            warm = ps.tile([65, 512], F32, tag="av", bufs=2, name="warm")
            for i in range(9):
                nc.tensor.matmul(warm[:], wstat[:, 0:65], wmov[:],
                                 start=True, stop=True)
            dummy_exp = sb.tile([1, 8], F32, tag="dummy_exp", name="dummy_exp")
            nc.scalar.activation(dummy_exp[:], wmov[0:1, 0:8],
                                 mybir.ActivationFunctionType.Exp, scale=1.0)

            # ---- loads: whole tiles (4KB contiguous runs), three rings in
            #      parallel, priority-ordered: the first alpha needs
            #      wq+xq0 (q proj sc0) then wk+xk0 (k proj sc0); xq1 next
            #      (qc1 half-alphas), then xk1, then everything v ----
            x_t = {}
            ring_i = [0]

            def x_ld(which, sc, eng):
                t = sb.tile([128, KT * 512], FP8, tag=f"x{which}{sc}",
                            name=f"x{which}_{sc}")
                x_t[(which, sc)] = t
                eng.dma_start(t[:], x_d[(which, sc)])

            def x3(which, sc):
                return x_t[(which, sc)][:].rearrange("p (k s) -> p k s", s=512)

            # Each DMA ring is a SERIAL queue (~110GB/s with 4KB runs); the
            # rings run in parallel.  Column-slicing a tensor halves the
            # DMA run size and the queue throughput with it, so tensors
            # split across two rings are split by PARTITION range (rows
            # stay 4KB-contiguous).  The scalar ring carries only bqk+xk0:
            # its descriptors occupy the ACT engine, which must be free
            # once the exp stream starts.  Emission order matters: the
            # first ~10 descriptors get fresh semaphores; later ones reuse
            # a slot and stall their issuing engine until a prior transfer
            # completes (harmless on sync/gpsimd, fatal on scalar).
            def w_ld_split(wt, wd, e0, e1):
                e0.dma_start(wt[0:64, :], wd[0:64, :])
                e1.dma_start(wt[64:128, :], wd[64:128, :])

            def x_ld_split(which, sc, e0, e1):
                t = sb.tile([128, KT * 512], FP8, tag=f"x{which}{sc}",
                            name=f"x{which}_{sc}")
                x_t[(which, sc)] = t
                e0.dma_start(t[0:64, :], x_d[(which, sc)][0:64, :])
                e1.dma_start(t[64:128, :], x_d[(which, sc)][64:128, :])

            x_ld("q", 0, nc.sync)                       # 1
            nc.scalar.dma_start(bqk_t[:], bqk_d)        # 2
            nc.gpsimd.dma_start(wq_t[:], w_d["q"])      # 3
            x_ld("k", 0, nc.scalar)                     # 4
            w_ld_split(wk_t, w_d["k"], nc.sync, nc.gpsimd)    # 5,6
            x_ld_split("q", 1, nc.sync, nc.gpsimd)            # 7,8
            x_ld_split("k", 1, nc.sync, nc.gpsimd)            # 9,10
            # zero the kz pads for o-tile 0: needed before the first alpha,
            # the gpsimd engine is free once its descriptors are out
            nc.gpsimd.memset(kz_t[0][0][64:128, :], 0.0)
            nc.gpsimd.memset(kz_t[0][1][0:64, :], 0.0)
            kz_zeroed.add(0)
            x_ld("v", 0, nc.sync)
            nc.gpsimd.dma_start(bv_t[:], bv_d)
            nc.gpsimd.dma_start(ones_t[:], ones_d[0:1, :])
            nc.sync.dma_start(ones64_t[:], ones_d[:, 0:KT * HL])
            nc.gpsimd.dma_start(wv_t[:], w_d["v"])
            x_ld("v", 1, nc.gpsimd)

            v4 = vp_t[:].rearrange("p (k n c) -> p k n c", n=HL, c=68)

            pp_live = {}

            def proj_qk(sc, ot, which, part=None, act_evac=False):
                """one o-tile, one s-chunk of the transposed q/k projection;
                part 0/1 emit half the DR chain each (fill-unit sizing),
                part None emits the whole group.  act_evac: for k, evacuate
                the h0 half on ACT (idle pre-stream) in parallel with DVE."""
                w_t = wq_t if which == "q" else wk_t
                w3 = w_t[:].rearrange("p (k o) -> p k o", o=GH)
                xv_ = x3(which, sc)
                if part != 1:
                    pp_live[(sc, ot, which)] = ps.tile(
                        [128, 1024], F32, tag="alpha", bufs=3,
                        name=f"pp{which}_{sc}_{ot}")
                pp = pp_live[(sc, ot, which)]
                kps = range(KT // 2) if part is None else (
                    range(2) if part == 0 else range(2, 4))
                for kp in kps:
                    nc.tensor.matmul(
                        pp[:, 0:512],
                        w3[:, 2 * kp:2 * kp + 2, ot * 128:(ot + 1) * 128],
                        xv_[:, 2 * kp:2 * kp + 2, :],
                        start=(kp == 0), stop=(kp == KT // 2 - 1),
                        perf_mode=DR)
                if part == 0:
                    return
                wi = 0 if which == "q" else 1
                bias = bqk_t[:, wi * OT + ot:wi * OT + ot + 1]
                ssl = slice(sc * 512, (sc + 1) * 512)
                if which == "q":
                    nc.vector.tensor_scalar(
                        qp_t[ot][:, ssl], pp[:, 0:512], bias, 0.0,
                        mybir.AluOpType.add, mybir.AluOpType.max)
                else:
                    for h in range(2):
                        pr = slice(h * 64, h * 64 + 64)
                        if h == 0 and act_evac:
                            # critical path: ACT is idle before the exp
                            # stream; do h0 there while DVE does h1
                            nc.scalar.activation(
                                kz_t[ot][h][pr, ssl], pp[pr, 0:512],
                                mybir.ActivationFunctionType.Relu,
                                bias=bias[pr, :], scale=1.0)
                        else:
                            nc.vector.tensor_scalar(
                                kz_t[ot][h][pr, ssl], pp[pr, 0:512],
                                bias[pr, :], 0.0,
                                mybir.AluOpType.add, mybir.AluOpType.max)
                pp_live.pop((sc, ot, which))

            def proj_v(sc, j, part=None):
                """one s-tile (128 rows of vp) within chunk sc"""
                st = sc * 4 + j
                wv3 = wv_t[:].rearrange("p (k o) -> p k o", o=GH)
                xv_ = x3("v", sc)
                if part != 1:
                    pp_live[("v", st)] = ps.tile([128, 1024], F32,
                                                 tag="alpha", bufs=3,
                                                 name=f"ppv_{st}")
                    nc.tensor.matmul(pp_live[("v", st)][:, 0:512],
                                     ones_t[:], bv_t[:],
                                     start=True, stop=False)
                pp = pp_live[("v", st)]
                kps = range(KT // 2) if part is None else (
                    range(2) if part == 0 else range(2, 4))
                for kp in kps:
                    nc.tensor.matmul(
                        pp[:, 0:512],
                        xv_[:, 2 * kp:2 * kp + 2, j * 128:(j + 1) * 128],
                        wv3[:, 2 * kp:2 * kp + 2, :],
                        start=False, stop=(kp == KT // 2 - 1),
                        perf_mode=DR)
                if part == 0:
                    return
                v3 = vp_t[:, st * VW8:(st + 1) * VW8].rearrange(
                    "p (n c) -> p n c", c=68)
                p3 = pp[:, 0:512].rearrange("p (n c) -> p n c", c=DH)
                nc.vector.tensor_scalar(
                    v3[:, :, 0:DH], p3, 0.0, None, mybir.AluOpType.max)
                pp_live.pop(("v", st))

            pt_all = {}
            fill_q = []

            def a_tile(n, k, qc=None):
                """alpha matmul(s) + exp for head n, k-tile k.  qc None:
                full [128,1024] tile (one exp).  qc 0/1: a HALF tile --
                only needs qp s-chunk qc, so the first exps can fire
                before xq1/q-proj(sc1) are done."""
                t, h = n // 2, n % 2
                pair, half = k // 2, k % 2
                pts = pt_all.setdefault(n, [None] * (KT // 2))
                if pts[pair] is None:
                    pts[pair] = sb.tile([128, 2048], FP8, tag="pt",
                                        bufs=32, name=f"pt_{n}_{pair}")
                cur = pts[pair]
                apt = ps.tile([128, 1024], F32, tag="alpha", bufs=3,
                              name=f"alp_{n}_{k}_{qc}")
                qcs = (0, 1) if qc is None else (qc,)
                for q_ in qcs:
                    nc.tensor.matmul(
                        apt[:, q_ * 512:(q_ + 1) * 512],
                        kz_t[t][h][:, k * 128:(k + 1) * 128],
                        qp_t[t][:, q_ * 512:(q_ + 1) * 512],
                        start=True, stop=True)
                if qc is None:
                    nc.scalar.activation(
                        cur[:, half * 1024:(half + 1) * 1024], apt[:],
                        mybir.ActivationFunctionType.Exp, scale=EXP_SCALE,
                        bias=expb_t[:])
                else:
                    off = half * 1024 + qc * 512
                    nc.scalar.activation(
                        cur[:, off:off + 512], apt[:, qc * 512:qc * 512 + 512],
                        mybir.ActivationFunctionType.Exp, scale=EXP_SCALE,
                        bias=expb_t[:])

            def alphas_h(n, pops=()):
                """alpha + exp for one head, full tiles, popping fill units
                from fill_q between tiles so the PE's ACT-limited stall
                time does the projections and AV chunks."""
                t = n // 2
                if t not in kz_zeroed:
                    kz_zeroed.add(t)
                    nc.gpsimd.memset(kz_t[t][0][64:128, :], 0.0)
                    nc.gpsimd.memset(kz_t[t][1][0:64, :], 0.0)
                for k in range(KT):
                    a_tile(n, k)
                    if k in pops and fill_q:
                        fill_q.pop(0)()

            hid_tiles = {}
            av_live = {}

            def avs_qc(n, qc, last=False, part=None):
                pts = pt_all[n]
                if qc == 0 and part != 1:
                    hid_tiles[n] = sb.tile([DH + 1, S], F32, tag="hid",
                                           bufs=3, name=f"hid_{n}")
                hid_t = hid_tiles[n]
                if part != 1:
                    av_live[(n, qc)] = ps.tile([DH + 1, 512], F32, tag="av",
                                               bufs=2, name=f"av_{n}_{qc}")
                av = av_live[(n, qc)]
                kps = range(KT // 2) if part is None else (
                    range(2) if part == 0 else range(2, 4))
                for kp in kps:
                    nc.tensor.matmul(
                        av[:],
                        v4[:, 2 * kp:2 * kp + 2, n, 0:DH + 1],
                        pts[kp][:].rearrange(
                            "p (k s) -> p k s",
                            s=1024)[:, :, qc * 512:(qc + 1) * 512],
                        start=(kp == 0), stop=(kp == KT // 2 - 1),
                        perf_mode=DR)
                if part == 0:
                    return
                av_live.pop((n, qc))
                if last:
                    # ACT is idle after its final exp — use it so the two
                    # tail evacuations run on different engines
                    nc.scalar.copy(
                        hid_t[:, qc * 512:(qc + 1) * 512], av[:])
                else:
                    nc.vector.tensor_copy(
                        hid_t[:, qc * 512:(qc + 1) * 512], av[:])
                # never the scalar ring: a DMA descriptor op there would
                # steal ~0.8us from the ACT exp stream
                if last:
                    # split the tail transfer across both rings so the run
                    # doesn't end waiting on one 130KB DMA
                    for ci, eng in ((0, nc.sync), (1, nc.gpsimd)):
                        cs = slice(qc * 512 + ci * 256, qc * 512 + ci * 256 + 256)
                        eng.dma_start(
                            hid_d[n * (DH + 1):(n + 1) * (DH + 1), cs],
                            hid_t[:, cs])
                else:
                    eng = nc.sync if ring_i[0] % 2 == 0 else nc.gpsimd
                    ring_i[0] += 1
                    eng.dma_start(
                        hid_d[n * (DH + 1):(n + 1) * (DH + 1),
                              qc * 512:(qc + 1) * 512],
                        hid_t[:, qc * 512:(qc + 1) * 512])
                if qc == 1:
                    pt_all.pop(n)
                    hid_tiles.pop(n)

            def u2(f, *a):
                fill_q.append(lambda: f(*a, part=0))
                fill_q.append(lambda: f(*a, part=1))

            # ---- early phase: minimize time to the first exp.  q(0,0)
            #      needs only wq+xq0 (first DMAs); k(0,0) needs wk+xk0
            #      (second wave).  The first four alpha tiles are qc0-only
            #      halves so they don't wait for xq1/q-proj(sc1). ----
            proj_qk(0, 0, "q")
            proj_qk(0, 0, "k", act_evac=True)
            a_tile(0, 0, qc=0)
            a_tile(0, 1, qc=0)
            a_tile(0, 2, qc=0)
            a_tile(0, 3, qc=0)
            proj_qk(1, 0, "q")
            a_tile(0, 0, qc=1)
            a_tile(0, 1, qc=1)
            a_tile(0, 2, qc=1)
            a_tile(0, 3, qc=1)
            proj_qk(1, 0, "k")
            a_tile(0, 4)
            a_tile(0, 5)
            # ones column of the AV stationary (needs ones64; first AV
            # unit pops much later, and the early DVE evacuations are done)
            nc.vector.tensor_copy(
                v4[:, :, :, DH:DH + 1],
                ones64_t[:].rearrange("p (k n one) -> p k n one", n=HL, one=1))
            proj_qk(0, 1, "q")
            a_tile(0, 6)
            a_tile(0, 7)

            # ---- steady state: head-at-a-time alpha/exp stream with fill
            #      units.  Each o-tile's four remaining projections drain
            #      two heads ahead of the alphas that read them; all eight
            #      v s-tiles drain before the first AV unit. ----
            u2(proj_qk, 0, 1, "k")
            u2(proj_qk, 1, 1, "q")
            u2(proj_qk, 1, 1, "k")
            alphas_h(1, pops=(1, 2, 3, 4, 5, 6))
            u2(proj_qk, 0, 2, "q")
            u2(proj_qk, 0, 2, "k")
            u2(proj_qk, 1, 2, "q")
            u2(proj_qk, 1, 2, "k")
            alphas_h(2, pops=(0, 1, 2, 3, 4, 5, 6, 7))
            u2(proj_v, 0, 0)
            u2(proj_v, 0, 1)
            u2(proj_v, 0, 2)
            u2(proj_v, 0, 3)
            alphas_h(3, pops=(0, 1, 2, 3, 4, 5, 6, 7))
            u2(proj_qk, 0, 3, "q")
            u2(proj_qk, 0, 3, "k")
            u2(proj_qk, 1, 3, "q")
            u2(proj_qk, 1, 3, "k")
            alphas_h(4, pops=(0, 1, 2, 3, 4, 5, 6, 7))
            u2(proj_v, 1, 0)
            u2(proj_v, 1, 1)
            u2(proj_v, 1, 2)
            u2(proj_v, 1, 3)
            alphas_h(5, pops=(0, 1, 2, 3, 4, 5, 6, 7))
            u2(avs_qc, 0, 0)
            u2(avs_qc, 0, 1)
            u2(avs_qc, 1, 0)
            u2(avs_qc, 1, 1)
            alphas_h(6, pops=(0, 1, 2, 3, 4, 5, 6, 7))
            u2(avs_qc, 2, 0)
            u2(avs_qc, 2, 1)
            u2(avs_qc, 3, 0)
            u2(avs_qc, 3, 1)
            u2(avs_qc, 4, 0)
            alphas_h(7, pops=(1, 2, 3, 4, 5, 6, 7))
            while fill_q:
                fill_q.pop(0)()
            avs_qc(4, 1)
            avs_qc(5, 0)
            avs_qc(5, 1)
            avs_qc(6, 0)
            avs_qc(6, 1)
            # head 7's first AV halves use pt pairs 0-1 (ready ~4 exps early);
            # only the two closing 2-DR chains depend on the final exp
            avs_qc(7, 0, part=0)
            avs_qc(7, 1, part=0)
            avs_qc(7, 0, part=1)
            avs_qc(7, 1, part=1, last=True)

    nc.compile()
    return nc


_NC_CACHE = {}


def _get_nc(mode):
    if mode not in _NC_CACHE:
        if mode != "fp8":
            raise ValueError(f"unsupported mode {mode}")
        _NC_CACHE[mode] = build_fp8()
    return _NC_CACHE[mode]


def _sbuf_layout_x(xT):
    """[H, S] transposed input -> per-chunk [128, KT*512] SBUF image"""
    x4 = xT.reshape(KT, 128, 2, 512)          # [k, p, sc, s]
    return [np.ascontiguousarray(
        x4[:, :, sc, :].transpose(1, 0, 2).reshape(128, KT * 512)).astype(E4)
        for sc in range(2)]


def _sbuf_layout_w(wT):
    """[H, GH] transposed weight -> [128, KT*GH] SBUF image"""
    w3 = wT.reshape(KT, 128, GH)
    return np.ascontiguousarray(
        w3.transpose(1, 0, 2).reshape(128, KT * GH)).astype(E4)


def _prep_inputs(inputs):
    q = np.asarray(inputs["query"], np.float32)
    k = np.asarray(inputs["key"], np.float32)
    v = np.asarray(inputs["value"], np.float32)
    Wq = np.asarray(inputs["Wq"], np.float32)
    Wk = np.asarray(inputs["Wk"], np.float32)
    Wv = np.asarray(inputs["Wv"], np.float32)
    bq = np.asarray(inputs["bq"], np.float32)
    bk = np.asarray(inputs["bk"], np.float32)
    bv = np.asarray(inputs["bv"], np.float32)

    xq = [_sbuf_layout_x(q[b].T) for b in range(B)]
    xk = [_sbuf_layout_x(k[b].T) for b in range(B)]
    xv = [_sbuf_layout_x(v[b].T) for b in range(B)]
    in_maps = []
    for c in range(NCORES):
        b, g = c // GROUPS, c % GROUPS
        sl = slice(g * GH, (g + 1) * GH)
        bqk = np.stack([(ESC * bq[sl]).reshape(OT, 128).T,
                        (ESC * bk[sl]).reshape(OT, 128).T],
                       1).reshape(128, 2 * OT)
        in_maps.append({
            "xq0": xq[b][0], "xq1": xq[b][1],
            "xk0": xk[b][0], "xk1": xk[b][1],
            "xv0": xv[b][0], "xv1": xv[b][1],
            "wq": _sbuf_layout_w(ESC * Wq[sl, :].T),
            "wk": _sbuf_layout_w(ESC * Wk[sl, :].T),
            "wv": _sbuf_layout_w(ESC * Wv[sl, :].T),
            "bqk": np.ascontiguousarray(bqk, dtype=np.float32),
            "bv": np.ascontiguousarray(ESC * bv[None, sl]).astype(E4),
            "onesd": np.ones((128, 128), E4),
        })
    return in_maps


def run(inputs, mode=MODE, trace=False):
    nc = _get_nc(mode)
    in_maps = _prep_inputs(inputs)
    res = bass_utils.run_bass_kernel_spmd(
        nc, in_maps, core_ids=list(range(NCORES)), trace=trace)

    masks = np.asarray(inputs["masks"], np.float32)
    query = np.asarray(inputs["query"], np.float32)
    out = np.empty((B, S, H), np.float32)
    for c in range(NCORES):
        b, g = c // GROUPS, c % GROUPS
        hid = res.results[c]["hid"].reshape(HL, DH + 1, S)
        hT = hid[:, :DH, :]                      # (HL, DH, S)  (32x scaled)
        se = hid[:, DH, :]                       # (HL, S)
        blk = (hT / (ESC * se[:, None, :])).transpose(2, 0, 1).reshape(S, GH)
        out[b, :, g * GH:(g + 1) * GH] = blk
    out = out * masks[:, :, None] + query
    return out, res


def kernel(**inputs) -> np.ndarray:
    out, _ = run(inputs)
    return out
